# revision 30
# baseline (speedup 1.0000x reference)
"""CompressedSparseAttention Trainium2 kernel (8 NeuronCores).

Sharding: data-parallel over batch (2) x tensor-parallel over head-pairs (4).
Core c handles batch b = c//4 and heads (2g, 2g+1) with g = c%4.
Each core computes its partial output  attn_out[:, hslice] @ wo[:, hslice].T
([2048, 512]) into a DRAM bounce buffer; an on-device grouped ReduceScatter
(groups [0-3], [4-7]) sums the 4 partials per batch and hands core c rows
[512g, 512(g+1)) of the final output, so each core only emits a [512, 512]
slice and the host gather is a pure concat.

Layouts inside a core (SBUF partition dim first):
  xT        [512, 2048]   x[b].T, 4 chunks of [128, 2048], fp32r
  qT/kT     [128, 2048]   rows = 2 heads x 64 dims, bf16 after RoPE
  k_cT      [128, 511]    compressed keys (dims on partitions)
  v_aug     16 x [128, 130]  v chunks transposed to [pos, dim] + ones cols
  vc_aug    4 x [128, 130]   v_c chunks transposed to [w, dim] + ones cols
  scores^T  [keys<=128, q]   PSUM; exp'd on ACT; masks via gpsimd affine_select
  av^T      [65, 512]     PSUM per (head, q-block): rows 0-63 = sum exp*v,
                          row 64 = sum exp (denominator via ones column)
"""

import math
import os

os.environ.setdefault("JAX_PLATFORMS", "axon,cpu")

import numpy as np

import concourse.bass as bass
import concourse.mybir as mybir
import concourse.tile as tile
from concourse import bacc
from concourse.bass import ds
from concourse.masks import make_identity

B = 2
L = 2048
D = 512
H = 8
HD = 64
RATIO = 8
STRIDE = 4
WINDOW = 128
THETA = 10000.0
LC = (L - RATIO) // STRIDE + 1  # 511
NCORES = 8
NB = L // 512  # 4 q-blocks of 512
NCH = L // 128  # 16 q-chunks of 128
KD = D // 128  # 4 contraction chunks

F32 = mybir.dt.float32
F32R = mybir.dt.float32r
BF16 = mybir.dt.bfloat16
AF = mybir.ActivationFunctionType
ALU = mybir.AluOpType

_CACHE = {}


def _build_nc(use_rs=True):
    nc = bacc.Bacc(
        "TRN2",
        target_bir_lowering=False,
        debug=False,
        num_devices=NCORES,
        name="csa" if use_rs else "csa_nors",
    )

    # DRAM I/O (per-core views). Inputs are kept small on purpose: the axon
    # client re-ships input buffers on every dispatch (~0.1ms/MB), so x and
    # the weights travel as bf16 and the RoPE tables as compact [32, L]
    # bases expanded on device.
    xq_d = nc.dram_tensor("xq", [L // 4, D], BF16, kind="ExternalInput")
    # 5 projection weights packed [512, 640]; cores c and c+4 are the same
    # head group, so each ships half and an AllGather over pairs [c, c+4]
    # reconstitutes the pack
    wpackh_d = nc.dram_tensor("wpackh", [D // 2, 640], BF16, kind="ExternalInput")
    woT_d = nc.dram_tensor("woT", [128, D], BF16, kind="ExternalInput")
    # rope tables: all cores need the same [64, L] cos/sin bases; each ships
    # 1/8 and an all-core AllGather rebuilds them
    trig8_d = nc.dram_tensor("trig8", [8, L], F32, kind="ExternalInput")
    sgn_d = nc.dram_tensor("sgn", [128, 1], F32, kind="ExternalInput")
    gate1_d = nc.dram_tensor("gate1", [1, RATIO], F32, kind="ExternalInput")
    sink2_d = nc.dram_tensor("sink2", [1, 2], F32, kind="ExternalInput")
    # bf16 output: halves the zero-output operand bytes shipped per dispatch
    # (the host upcasts); one extra rounding, well within the error budget
    outp_d = nc.dram_tensor(
        "outp",
        [L // 4 if use_rs else L, D],
        BF16 if use_rs else F32,
        kind="ExternalOutput",
    )

    with tile.TileContext(nc) as tc:
        with tc.tile_pool(name="consts", bufs=1) as cp, \
             tc.tile_pool(name="work", bufs=1) as wp, \
             tc.tile_pool(name="dram", bufs=1, space="DRAM") as dp, \
             tc.tile_pool(name="ps", bufs=7, space="PSUM") as pp, \
             tc.tile_pool(name="pss", bufs=1, space="PSUM") as pps:

            if use_rs:
                part = dp.tile([L, D], F32, tag="part")
                rs_out = dp.tile([L // 4, D], F32, tag="rs_out")
            else:
                part = outp_d

            # ---------------- init: gather x across the 4 cores of a batch;
            # each core ships only its quarter of x (position-major)
            xq_b = dp.tile([L // 4, D], BF16, tag="xq_b")
            xg = dp.tile([L, D], BF16, tag="xg")
            nc.gpsimd.dma_start(xq_b[:, :], xq_d[:, :])
            nc.gpsimd.collective_compute(
                "AllGather",
                mybir.AluOpType.bypass,
                replica_groups=[[0, 1, 2, 3], [4, 5, 6, 7]],
                ins=[xq_b.opt()],
                outs=[xg.opt()],
            )

            trig8_b = dp.tile([8, L], F32, tag="trig8_b")
            trig_g = dp.tile([64, L], F32, tag="trig_g")
            nc.gpsimd.dma_start(trig8_b[:, :], trig8_d[:, :])
            nc.gpsimd.collective_compute(
                "AllGather",
                mybir.AluOpType.bypass,
                replica_groups=[[0, 1, 2, 3, 4, 5, 6, 7]],
                ins=[trig8_b.opt()],
                outs=[trig_g.opt()],
            )

            wpack_b = dp.tile([D // 2, 640], BF16, tag="wpack_b")
            wpack_g = dp.tile([D, 640], BF16, tag="wpack_g")
            nc.gpsimd.dma_start(wpack_b[:, :], wpackh_d[:, :])
            nc.gpsimd.collective_compute(
                "AllGather",
                mybir.AluOpType.bypass,
                replica_groups=[[0, 4], [1, 5], [2, 6], [3, 7]],
                ins=[wpack_b.opt()],
                outs=[wpack_g.opt()],
            )

            # ---------------- init: DMA constants ----------------
            xT = [
                cp.tile([128, L], BF16, tag=f"xt{c}", name=f"xt{c}")
                for c in range(KD)
            ]

            def load_w(j, tag):
                w = []
                for c in range(KD):
                    t = cp.tile([128, 128], BF16, tag=f"{tag}{c}", name=f"{tag}{c}")
                    nc.sync.dma_start(
                        out=t, in_=wpack_g[ds(128 * c, 128), ds(128 * j, 128)]
                    )
                    w.append(t)
                return w

            wq = load_w(0, "wq")
            wk = load_w(1, "wk")
            wv = load_w(2, "wv")
            wkc = load_w(3, "wkc")
            wvc = load_w(4, "wvc")

            woT_bf = cp.tile([128, D], BF16, tag="woT_bf")
            nc.sync.dma_start(out=woT_bf, in_=woT_d[:, :])

            # expand [32, L] cos/sin bases to the [128, L] working tables:
            # row r uses frequency r%32; sin rows are signed -,+,-,+ per
            # 32-row quarter (the rotate-half layout of rope_block)
            cos32 = cp.tile([32, L], F32, tag="cos32")
            nc.sync.dma_start(out=cos32, in_=trig_g[0:32, :])
            sin32 = cp.tile([32, L], F32, tag="sin32")
            nc.sync.dma_start(out=sin32, in_=trig_g[32:64, :])
            sgn = cp.tile([128, 1], F32, tag="sgn")
            nc.sync.dma_start(out=sgn, in_=sgn_d[:, :])
            cosT = cp.tile([128, L], F32, tag="cosT")
            sinT = cp.tile([128, L], F32, tag="sinT")
            for q4 in range(4):
                nc.gpsimd.tensor_copy(out=cosT[ds(32 * q4, 32), :], in_=cos32)
                nc.gpsimd.tensor_copy(out=sinT[ds(32 * q4, 32), :], in_=sin32)
            sinST = cp.tile([128, L], F32, tag="sinST")
            nc.vector.tensor_scalar(
                out=sinST, in0=sinT, scalar1=sgn, scalar2=None, op0=ALU.mult
            )
            gate1 = cp.tile([1, RATIO], F32, tag="gate1")
            nc.sync.dma_start(out=gate1, in_=gate1_d[:, :])
            gateb = cp.tile([128, RATIO], F32, tag="gateb")
            nc.gpsimd.partition_broadcast(gateb, gate1)
            sink2 = cp.tile([1, 2], F32, tag="sink2")
            nc.sync.dma_start(out=sink2, in_=sink2_d[:, :])

            # exp(sink) broadcast to all partitions
            exps = cp.tile([1, 2], F32, tag="exps")
            nc.scalar.activation(out=exps, in_=sink2, func=AF.Exp)
            expsb = cp.tile([128, 2], F32, tag="expsb")
            nc.gpsimd.partition_broadcast(expsb, exps)

            # identities for PE transpose
            ident_bf = cp.tile([128, 128], BF16, tag="ident_bf")
            make_identity(nc, ident_bf)
            ident_f = cp.tile([128, 128], F32, tag="ident_f")
            make_identity(nc, ident_f)

            # reconstruct xT tiles [128 dims, L pos] from gathered xg [L, D]
            for ch in range(NCH):
                t_pm = wp.tile([128, D], BF16, tag="t_pm", bufs=2, name="t_pm")
                nc.sync.dma_start(out=t_pm, in_=xg[ds(128 * ch, 128), :])
                for c in range(KD):
                    tp = pp.tile([128, 128], BF16, tag="bank", name="xg_tr")
                    nc.tensor.transpose(tp, t_pm[:, ds(128 * c, 128)], ident_bf)
                    nc.vector.tensor_copy(
                        out=xT[c][:, ds(128 * ch, 128)], in_=tp
                    )

            # ---------------- P1: projections + RoPE + pooling ----------------
            qT = cp.tile([128, L], BF16, tag="qT")
            kT = cp.tile([128, L], BF16, tag="kT")
            vT_bf = cp.tile([128, L], BF16, tag="vT_bf")
            y_kc = cp.tile([128, L], F32, tag="y_kc")
            y_vc = cp.tile([128, L], F32, tag="y_vc")

            def project(wlist, qb):
                ps = pp.tile([128, 512], F32, tag="bank", name="proj_ps")
                for c in range(KD):
                    nc.tensor.matmul(
                        ps,
                        wlist[c],
                        xT[c][:, ds(512 * qb, 512)],
                        start=(c == 0),
                        stop=(c == KD - 1),
                    )
                return ps

            def rope_block(ps, outT, qb):
                qraw = wp.tile([128, 512], F32, tag="qraw", bufs=2, name="qraw")
                nc.scalar.copy(out=qraw, in_=ps)
                qsw = wp.tile([128, 512], F32, tag="qsw", bufs=2, name="qsw")
                for a, bb in ((0, 32), (32, 0), (64, 96), (96, 64)):
                    nc.gpsimd.tensor_copy(
                        out=qsw[ds(a, 32), :], in_=qraw[ds(bb, 32), :]
                    )
                m1 = wp.tile([128, 512], F32, tag="m1", bufs=2, name="m1")
                nc.vector.tensor_mul(m1, ps, cosT[:, ds(512 * qb, 512)])
                m2 = wp.tile([128, 512], F32, tag="m2", bufs=2, name="m2")
                nc.vector.tensor_mul(m2, qsw, sinST[:, ds(512 * qb, 512)])
                nc.vector.tensor_add(outT[:, ds(512 * qb, 512)], m1, m2)

            for qb in range(NB):
                ps = project(wq, qb)
                rope_block(ps, qT, qb)
            for qb in range(NB):
                ps = project(wk, qb)
                rope_block(ps, kT, qb)
            for qb in range(NB):
                ps = project(wv, qb)
                nc.scalar.copy(out=vT_bf[:, ds(512 * qb, 512)], in_=ps)
            for qb in range(NB):
                ps = project(wkc, qb)
                nc.scalar.copy(out=y_kc[:, ds(512 * qb, 512)], in_=ps)
            for qb in range(NB):
                ps = project(wvc, qb)
                nc.scalar.copy(out=y_vc[:, ds(512 * qb, 512)], in_=ps)

            # pooling: kc/vc[dim, w] = sum_r gate[r] * y[dim, 4w + r]
            def pool(y, out_bf):
                y4 = y.rearrange("p (w r) -> p r w", r=STRIDE)
                acc = [
                    wp.tile([128, LC], F32, tag="poolA", bufs=1, name="poolA"),
                    wp.tile([128, LC], F32, tag="poolB", bufs=1, name="poolB"),
                ]
                nc.vector.tensor_scalar(
                    out=acc[0],
                    in0=y4[:, 0, 0:LC],
                    scalar1=gateb[:, 0:1],
                    scalar2=None,
                    op0=ALU.mult,
                )
                for r in range(1, RATIO):
                    dst = out_bf if r == RATIO - 1 else acc[r % 2]
                    nc.vector.scalar_tensor_tensor(
                        out=dst,
                        in0=y4[:, r % STRIDE, (r // STRIDE):(r // STRIDE) + LC],
                        scalar=gateb[:, ds(r, 1)],
                        in1=acc[(r - 1) % 2],
                        op0=ALU.mult,
                        op1=ALU.add,
                    )

            k_cT = cp.tile([128, LC], BF16, tag="k_cT")
            v_cT = cp.tile([128, LC], BF16, tag="v_cT")
            pool(y_kc, k_cT)
            pool(y_vc, v_cT)

            # transpose v -> v_aug chunks [pos, dim] (+ones col at 64 and 129)
            v_aug = []
            for ch in range(NCH):
                va = cp.tile([128, 130], BF16, tag=f"v_aug{ch}", name=f"v_aug{ch}")
                nc.gpsimd.memset(va, 1.0)
                tp = pps.tile([128, 128], BF16, tag="small", name="tr_ps")
                nc.tensor.transpose(tp, vT_bf[:, ds(128 * ch, 128)], ident_bf)
                nc.vector.tensor_copy(out=va[:, 0:64], in_=tp[:, 0:64])
                nc.vector.tensor_copy(out=va[:, 65:129], in_=tp[:, 64:128])
                v_aug.append(va)

            vc_aug = []
            for ch in range(4):
                wlen = min(128, LC - 128 * ch)  # 128,128,128,127
                va = cp.tile([128, 130], BF16, tag=f"vc_aug{ch}", name=f"vc_aug{ch}")
                nc.gpsimd.memset(va, 1.0)
                tp = pps.tile([128, 128], BF16, tag="small", name="trc_ps")
                nc.tensor.transpose(
                    tp[0:wlen, :], v_cT[:, ds(128 * ch, wlen)], ident_bf
                )
                nc.vector.tensor_copy(out=va[0:wlen, 0:64], in_=tp[0:wlen, 0:64])
                nc.vector.tensor_copy(out=va[0:wlen, 65:129], in_=tp[0:wlen, 64:128])
                vc_aug.append(va)

            # ---------------- P2: attention ----------------
            rec = [cp.tile([128, NCH], F32, tag=f"rec{h}", name=f"rec{h}") for h in range(2)]
            avT = []  # [128, 512] bf16 per q-block: rows 0-63 h0, 64-127 h1
            for qb in range(NB):
                at = cp.tile([128, 512], BF16, tag=f"avT{qb}", name=f"avT{qb}")
                avT.append(at)

            for qb in range(NB):
                for h in range(2):
                    hs = 64 * h
                    qs = qT[ds(hs, 64), ds(512 * qb, 512)]
                    av = pp.tile([65, 512], F32, tag="bank", name=f"av_{qb}_{h}")
                    first_av = [True]

                    def av_mm(lhsT, rhs, cols, stop=False):
                        nc.tensor.matmul(
                            av[:, cols] if cols is not None else av,
                            lhsT,
                            rhs,
                            start=first_av[0],
                            stop=stop,
                            skip_group_check=True,
                        )
                        first_av[0] = False

                    # --- compressed branch ---
                    for wc in range(qb + 1):
                        wlen = min(128, LC - 128 * wc)
                        sc = pp.tile([128, 512], F32, tag="bank", name="sc_ps")
                        nc.tensor.matmul(
                            sc[0:wlen, :],
                            k_cT[ds(hs, 64), ds(128 * wc, wlen)],
                            qs,
                            start=True,
                            stop=True,
                        )
                        ex = wp.tile([128, 512], BF16, tag="exc", bufs=3, name="exc")
                        nc.scalar.activation(
                            out=ex[0:wlen, :], in_=sc[0:wlen, :], func=AF.Exp,
                            scale=0.125,
                        )
                        if wc >= qb - 1:
                            # causal: keep q_rel >= 4*w_rel + 7 - 512*(qb - wc)
                            nc.gpsimd.affine_select(
                                out=ex[0:wlen, :],
                                in_=ex[0:wlen, :],
                                compare_op=ALU.is_ge,
                                fill=0.0,
                                base=-7 + 512 * (qb - wc),
                                pattern=[[1, 512]],
                                channel_multiplier=-4,
                            )
                        av_mm(
                            vc_aug[wc][0:wlen, ds(65 * h, 65)],
                            ex[0:wlen, :],
                            None,
                        )

                    # --- local window branch ---
                    for sub in range(4):
                        c = 4 * qb + sub
                        qcs = qT[ds(hs, 64), ds(128 * c, 128)]
                        wps = pp.tile([128, 256], F32, tag="bank", name="win_ps")
                        if c > 0:
                            nc.tensor.matmul(
                                wps[:, 0:128],
                                kT[ds(hs, 64), ds(128 * (c - 1), 128)],
                                qcs,
                                start=True,
                                stop=True,
                                skip_group_check=True,
                            )
                        nc.tensor.matmul(
                            wps[:, 128:256],
                            kT[ds(hs, 64), ds(128 * c, 128)],
                            qcs,
                            start=True,
                            stop=True,
                            skip_group_check=True,
                        )
                        exw = wp.tile([128, 256], BF16, tag="exw", bufs=3, name="exw")
                        lo = 0 if c > 0 else 128
                        nc.scalar.activation(
                            out=exw[:, lo:256], in_=wps[:, lo:256], func=AF.Exp,
                            scale=0.125,
                        )
                        if c > 0:
                            # prev chunk: keep k_rel > q_rel
                            nc.gpsimd.affine_select(
                                out=exw[:, 0:128],
                                in_=exw[:, 0:128],
                                compare_op=ALU.is_gt,
                                fill=0.0,
                                base=0,
                                pattern=[[-1, 128]],
                                channel_multiplier=1,
                            )
                        # current chunk: keep q_rel >= k_rel
                        nc.gpsimd.affine_select(
                            out=exw[:, 128:256],
                            in_=exw[:, 128:256],
                            compare_op=ALU.is_ge,
                            fill=0.0,
                            base=0,
                            pattern=[[1, 128]],
                            channel_multiplier=-1,
                        )
                        cols = ds(128 * sub, 128)
                        if c > 0:
                            av_mm(
                                v_aug[c - 1][:, ds(65 * h, 65)], exw[:, 0:128], cols
                            )
                        av_mm(
                            v_aug[c][:, ds(65 * h, 65)], exw[:, 128:256], cols,
                            stop=(sub == 3),
                        )

                    # --- denominator -> reciprocal in [q, 1] layout ---
                    drow = wp.tile([1, 512], F32, tag="drow", bufs=2, name="drow")
                    nc.scalar.copy(out=drow, in_=av[64:65, :])
                    dcol = pps.tile([128, 4], F32, tag="small", name="dcol")
                    for c4 in range(4):
                        nc.tensor.transpose(
                            dcol[:, ds(c4, 1)],
                            drow[:, ds(128 * c4, 128)],
                            ident_f[0:1, 0:1],
                        )
                    dsb = wp.tile([128, 4], F32, tag="dsb", bufs=2, name="dsb")
                    nc.vector.tensor_scalar(
                        out=dsb, in0=dcol, scalar1=expsb[:, ds(h, 1)], scalar2=None,
                        op0=ALU.add,
                    )
                    nc.vector.reciprocal(
                        out=rec[h][:, ds(4 * qb, 4)], in_=dsb
                    )

                    # numerator rows -> SBUF (bf16) for the wo matmul
                    nc.scalar.copy(
                        out=avT[qb][ds(hs, 64), :], in_=av[0:64, :]
                    )

            # ---------------- P3: output projection + normalize ----------------
            for qb in range(NB):
                for sub in range(4):
                    c = 4 * qb + sub
                    wo0 = pp.tile([128, 512], F32, tag="bank", name="wo0")
                    nc.tensor.matmul(
                        wo0, avT[qb][0:64, ds(128 * sub, 128)], woT_bf[0:64, :],
                        start=True, stop=True,
                    )
                    wo1 = pp.tile([128, 512], F32, tag="bank", name="wo1")
                    nc.tensor.matmul(
                        wo1, avT[qb][64:128, ds(128 * sub, 128)], woT_bf[64:128, :],
                        start=True, stop=True,
                    )
                    t0 = wp.tile([128, 512], F32, tag="t0", bufs=2, name="t0")
                    nc.scalar.activation(
                        out=t0, in_=wo0, func=AF.Copy, scale=rec[0][:, ds(c, 1)]
                    )
                    osb = wp.tile([128, 512], F32, tag="osb", bufs=3, name="osb")
                    nc.vector.scalar_tensor_tensor(
                        out=osb,
                        in0=wo1,
                        scalar=rec[1][:, ds(c, 1)],
                        in1=t0,
                        op0=ALU.mult,
                        op1=ALU.add,
                    )
                    nc.sync.dma_start(out=part[ds(128 * c, 128), :], in_=osb)

            if use_rs:
                # tensor-parallel sum over the 4 cores of each batch; core at
                # group position g receives rows [512g, 512(g+1))
                nc.gpsimd.collective_compute(
                    "ReduceScatter",
                    mybir.AluOpType.add,
                    replica_groups=[[0, 1, 2, 3], [4, 5, 6, 7]],
                    ins=[part.opt()],
                    outs=[rs_out.opt()],
                )
                for q4 in range(4):
                    ob = wp.tile([128, D], F32, tag="ob", bufs=2, name="ob")
                    nc.sync.dma_start(out=ob, in_=rs_out[ds(128 * q4, 128), :])
                    obh = wp.tile([128, D], BF16, tag="obh", bufs=2, name="obh")
                    nc.scalar.copy(out=obh, in_=ob)
                    nc.sync.dma_start(out=outp_d[ds(128 * q4, 128), :], in_=obh)

    nc.compile()
    return nc


def _host_prep(inputs):
    """Build the 8 per-core input maps from full inputs."""
    x = np.asarray(inputs["x"], dtype=np.float32)
    wq = np.asarray(inputs["wq"], dtype=np.float32)
    wk = np.asarray(inputs["wk"], dtype=np.float32)
    wv = np.asarray(inputs["wv"], dtype=np.float32)
    wo = np.asarray(inputs["wo"], dtype=np.float32)
    wk_c = np.asarray(inputs["wk_c"], dtype=np.float32)
    wv_c = np.asarray(inputs["wv_c"], dtype=np.float32)
    gate_logits = np.asarray(inputs["gate_logits"], dtype=np.float32)
    sink_logit = np.asarray(inputs["sink_logit"], dtype=np.float32)

    bf16 = mybir.dt.np(BF16)

    # rope tables: compact [32, L] bases; the kernel expands them on device
    half = HD // 2
    inv_freq = 1.0 / (THETA ** (np.arange(half, dtype=np.float32) / half))
    t = np.arange(L, dtype=np.float32)
    f = t[:, None] * inv_freq[None, :]  # [L, 32]
    cos32 = np.ascontiguousarray(np.cos(f).T.astype(np.float32))  # [32, L]
    sin32 = np.ascontiguousarray(np.sin(f).T.astype(np.float32))
    sgn = np.repeat(np.array([-1.0, 1.0, -1.0, 1.0], np.float32), 32)[:, None]
    sgn = np.ascontiguousarray(sgn)

    gv = np.exp(gate_logits - gate_logits.max())
    gate1 = (gv / gv.sum()).astype(np.float32)[None, :]

    trig64 = np.vstack([cos32, sin32])  # [64, L]
    xq_by_batch = [
        [
            np.ascontiguousarray(x[b, 512 * g : 512 * (g + 1), :]).astype(bf16)
            for g in range(4)
        ]
        for b in range(B)
    ]
    # [512, 640] packed projection weights per head group (5 x 128 columns)
    packs = [
        np.concatenate(
            [
                w[128 * grp : 128 * (grp + 1), :].T
                for w in (wq, wk, wv, wk_c, wv_c)
            ],
            axis=1,
        ).astype(bf16)
        for grp in range(4)
    ]
    in_maps = []
    for core in range(NCORES):
        b, grp = divmod(core, 4)
        sl = slice(128 * grp, 128 * (grp + 1))
        half = slice(256 * (core // 4), 256 * (core // 4) + 256)
        in_maps.append(
            {
                "xq": xq_by_batch[b][grp],
                "wpackh": np.ascontiguousarray(packs[grp][half]),
                "woT": wo[:, sl].T.astype(bf16),
                "trig8": np.ascontiguousarray(trig64[8 * core : 8 * (core + 1)]),
                "sgn": sgn,
                "gate1": gate1,
                "sink2": np.ascontiguousarray(
                    sink_logit[2 * grp : 2 * grp + 2, 0][None, :]
                ),
            }
        )
    return in_maps


def _get_exec():
    """Build (once) and cache the jitted 8-core PJRT executable.

    A single executable per process is mandatory: the program contains a
    collective, and dispatching a second PJRT executable of it desyncs the
    axon mesh. kernel() and any timing harness must share this one.
    """
    if "exec" in _CACHE:
        return _CACHE["exec"]

    import jax
    from jax.sharding import Mesh, PartitionSpec
    from jax.experimental.shard_map import shard_map
    from concourse import bass2jax

    bass2jax.install_neuronx_cc_hook()
    nc = _CACHE.get("nc")
    if nc is None:
        nc = _CACHE["nc"] = _build_nc()

    partition_name = nc.partition_id_tensor.name if nc.partition_id_tensor else None
    in_names, out_names, out_avals, zero_outs = [], [], [], []
    for alloc in nc.m.functions[0].allocations:
        if not isinstance(alloc, mybir.MemoryLocationSet):
            continue
        name = alloc.memorylocations[0].name
        if alloc.kind == "ExternalInput":
            if name != partition_name:
                in_names.append(name)
        elif alloc.kind == "ExternalOutput":
            shape = tuple(alloc.tensor_shape)
            dtype = mybir.dt.np(alloc.dtype)
            out_avals.append(jax.core.ShapedArray(shape, dtype))
            zero_outs.append(np.zeros(shape, dtype))
            out_names.append(name)
    n_params = len(in_names)
    all_in_names = tuple(
        in_names + out_names + ([partition_name] if partition_name else [])
    )

    def _body(*args):
        operands = list(args)
        if partition_name is not None:
            operands.append(bass2jax.partition_id_tensor())
        outs = bass2jax._bass_exec_p.bind(
            *operands,
            out_avals=tuple(out_avals),
            in_names=all_in_names,
            out_names=tuple(out_names),
            lowering_input_output_aliases=(),
            sim_require_finite=True,
            sim_require_nnan=True,
            nc=nc,
        )
        return tuple(outs)

    devices = jax.devices("axon")[:NCORES]
    mesh = Mesh(np.asarray(devices), ("core",))
    in_specs = (PartitionSpec("core"),) * (n_params + len(out_names))
    out_specs = (PartitionSpec("core"),) * len(out_names)
    sharded = jax.jit(
        shard_map(_body, mesh=mesh, in_specs=in_specs, out_specs=out_specs,
                  check_rep=False),
        keep_unused=True,
    )
    st = {
        "nc": nc,
        "sharded": sharded,
        "in_names": in_names,
        "out_names": out_names,
        "out_avals": out_avals,
        "zero_outs": zero_outs,
    }
    _CACHE["exec"] = st
    return st


def _prepare_args(inputs):
    """Host-prep + device_put the concatenated per-core args."""
    import jax

    st = _get_exec()
    in_maps = _host_prep(inputs)
    per_core = [[np.asarray(m[name]) for name in st["in_names"]] for m in in_maps]
    concat_in = [
        np.concatenate([per_core[c][i] for c in range(NCORES)], axis=0)
        for i in range(len(st["in_names"]))
    ]
    concat_zeros = [
        np.zeros((NCORES * z.shape[0], *z.shape[1:]), z.dtype)
        for z in st["zero_outs"]
    ]
    return [jax.device_put(a) for a in concat_in + concat_zeros]


def _run(args):
    """One dispatch of the cached executable; returns the jax output arrays."""
    st = _get_exec()
    return st["sharded"](*args)


def kernel(**inputs) -> np.ndarray:
    st = _get_exec()
    args = _prepare_args(inputs)
    out_arrs = _run(args)
    res = np.asarray(out_arrs[0]).reshape(NCORES, L // 4, D)
    out = np.zeros((B, L, D), dtype=np.float32)
    for core in range(NCORES):
        b, g = divmod(core, 4)
        out[b, 512 * g : 512 * (g + 1)] = res[core].astype(np.float32)
    return out

